# revision 4
# baseline (speedup 1.0000x reference)
"""Mixtral MoE layer (top-2 of 8 experts) as a Trainium2 Bass/Tile kernel.

Strategy (expert-parallel, per the original tp_size/expert_indices code):
  - 8 NeuronCores, one expert per core. Gate is replicated.
  - Host pre-transposes x -> xT [H,T] and weights -> w1t/w3t [H,F], w2t [F,H]
    so every SBUF tile load is a natural contiguous 2D slice (no on-device
    transposes anywhere).
  - On device, each core:
      * computes router logits for all tokens (xT-tile as stationary operand,
        gate.T as moving), softmax + top-2 + renorm along the free dim; the
        softmax denominator cancels in the top-2 renormalization, so the
        per-token weight for expert e is exp_e * [exp_e >= v2] / (v1 + v2).
      * runs its expert's MLP densely over all tokens in transposed space:
        h1T/h3T [F-part, T-col] = w1t/w3t-tile.T @ xT, g = silu(h1T)*h3T,
        out [T-part, H-col] = gT-tile.T @ w2t  (g is already in the exact
        stationary layout GEMM3 wants).
      * scales out rows by its expert's per-token routing weight (a
        per-partition scalar in this layout) and writes a [T,H] partial.
  - Host sums the 8 partials (the "all-reduce") and reshapes to [B,S,H].
  - Per-core column-0 trick: each core's gate copy has its columns permuted
    so that column 0 is its own expert -> one SPMD program, no dynamic
    indexing.

Tiling: tokens in chunks of TC=1024; F processed in NQ=4 quarters with an
SBUF fp32 accumulator for the GEMM3 partial sums so that the g-chunk and the
w2 tiles both stay small enough for SBUF.
"""

from contextlib import ExitStack

import numpy as np

import concourse.bacc as bacc
import concourse.tile as tile
from concourse import mybir
from concourse.bass_utils import run_bass_kernel_spmd

P = 128
AF = mybir.ActivationFunctionType
OP = mybir.AluOpType
AX = mybir.AxisListType
F32 = mybir.dt.float32


def build_moe_nc(T=4096, H=1024, F=3584, E=8, TC=1024, NQ=4, silu_native=True):
    """Build the single-core SPMD program. Returns the compiled Bacc."""
    HT = H // P          # contraction k-tiles for GEMM1/2
    FT = F // P          # f tiles
    FQ = FT // NQ        # f-tiles per quarter
    NCH = T // TC        # token chunks
    NW = min(512, TC)    # moving free dim per matmul (fp32 max 512)
    TH = TC // NW        # moving-operand slices per chunk
    NTT = TC // P        # 128-token tiles per chunk
    HW = min(512, H)     # output H column group width
    HC = H // HW
    assert FT % NQ == 0 and T % TC == 0 and TC % P == 0 and H % HW == 0

    nc = bacc.Bacc("TRN2", target_bir_lowering=False, debug=False)
    xt_d = nc.dram_tensor("xt", [H, T], F32, kind="ExternalInput").ap()
    gwt_d = nc.dram_tensor("gwt", [H, E], F32, kind="ExternalInput").ap()
    w1t_d = nc.dram_tensor("w1t", [H, F], F32, kind="ExternalInput").ap()
    w3t_d = nc.dram_tensor("w3t", [H, F], F32, kind="ExternalInput").ap()
    w2t_d = nc.dram_tensor("w2t", [F, H], F32, kind="ExternalInput").ap()
    out_d = nc.dram_tensor("out", [T, H], F32, kind="ExternalOutput").ap()

    with tile.TileContext(nc) as tc, ExitStack() as ctx:
        xt_pool = ctx.enter_context(tc.tile_pool(name="xt", bufs=HT))
        gw_pool = ctx.enter_context(tc.tile_pool(name="gw", bufs=HT))
        w13_pool = ctx.enter_context(tc.tile_pool(name="w13", bufs=2 * HT))
        w2_pool = ctx.enter_context(tc.tile_pool(name="w2", bufs=FQ * HC))
        g_pool = ctx.enter_context(tc.tile_pool(name="g", bufs=FQ))
        acc_pool = ctx.enter_context(tc.tile_pool(name="acc", bufs=NTT * HC))
        tmp_pool = ctx.enter_context(tc.tile_pool(name="tmp", bufs=4))
        rt_pool = ctx.enter_context(tc.tile_pool(name="rt", bufs=16))
        sc_pool = ctx.enter_context(tc.tile_pool(name="sc", bufs=2 * NTT))
        ob_pool = ctx.enter_context(tc.tile_pool(name="ob", bufs=4))
        ps12 = ctx.enter_context(tc.tile_pool(name="ps12", bufs=6, space="PSUM"))
        ps3 = ctx.enter_context(tc.tile_pool(name="ps3", bufs=2, space="PSUM"))

        # gate weights resident for the whole kernel
        gw_tiles = []
        for h in range(HT):
            t_ = gw_pool.tile([P, E], F32, tag="gw")
            nc.sync.dma_start(out=t_[:], in_=gwt_d[h * P:(h + 1) * P, :])
            gw_tiles.append(t_)

        for c in range(NCH):
            t0 = c * TC
            # ---- xT chunk [H, TC] as HT tiles of [P, TC]
            xts = []
            for h in range(HT):
                t_ = xt_pool.tile([P, TC], F32, tag="xt")
                nc.sync.dma_start(out=t_[:], in_=xt_d[h * P:(h + 1) * P, t0:t0 + TC])
                xts.append(t_)

            # ---- routing for this chunk: per 128-token tile
            scales = []
            for tt in range(NTT):
                psr = ps3.tile([P, E], F32, tag="ps3")
                for h in range(HT):
                    nc.tensor.matmul(
                        psr[:], xts[h][:, tt * P:(tt + 1) * P], gw_tiles[h][:],
                        start=(h == 0), stop=(h == HT - 1))
                lg = rt_pool.tile([P, E], F32, tag="rt")
                nc.scalar.copy(lg[:], psr[:])
                nm = rt_pool.tile([P, 1], F32, tag="rt")
                nc.vector.tensor_reduce(nm[:], lg[:], AX.X, OP.max, negate=True)
                ex = rt_pool.tile([P, E], F32, tag="rt")
                nc.scalar.activation(ex[:], lg[:], AF.Exp, bias=nm[:])
                v1 = rt_pool.tile([P, 1], F32, tag="rt")
                nc.vector.tensor_reduce(v1[:], ex[:], AX.X, OP.max)
                ltm = rt_pool.tile([P, E], F32, tag="rt")
                nc.vector.tensor_scalar(ltm[:], ex[:], v1[:], None, OP.is_lt)
                e2 = rt_pool.tile([P, E], F32, tag="rt")
                nc.vector.tensor_tensor(e2[:], ex[:], ltm[:], OP.mult)
                v2 = rt_pool.tile([P, 1], F32, tag="rt")
                nc.vector.tensor_reduce(v2[:], e2[:], AX.X, OP.max)
                den = rt_pool.tile([P, 1], F32, tag="rt")
                nc.vector.tensor_tensor(den[:], v1[:], v2[:], OP.add)
                rd = rt_pool.tile([P, 1], F32, tag="rt")
                nc.vector.reciprocal(rd[:], den[:])
                # column 0 is this core's expert; weight = ex0*[ex0>=v2]/(v1+v2)
                ge = rt_pool.tile([P, 1], F32, tag="rt")
                nc.vector.tensor_scalar(ge[:], ex[:, 0:1], v2[:], None, OP.is_ge)
                w0 = rt_pool.tile([P, 1], F32, tag="rt")
                nc.vector.tensor_tensor(w0[:], ex[:, 0:1], ge[:], OP.mult)
                sc = sc_pool.tile([P, 1], F32, tag="sc")
                nc.vector.tensor_tensor(sc[:], w0[:], rd[:], OP.mult)
                scales.append(sc)

            acc_tiles = {}
            for q in range(NQ):
                # ---- quarter weight loads: w1/w3 [P, FQ*P] per h-tile
                w1q, w3q = [], []
                for h in range(HT):
                    t1 = w13_pool.tile([P, FQ * P], F32, tag="w13")
                    nc.sync.dma_start(
                        out=t1[:],
                        in_=w1t_d[h * P:(h + 1) * P, q * FQ * P:(q + 1) * FQ * P])
                    w1q.append(t1)
                    t3 = w13_pool.tile([P, FQ * P], F32, tag="w13")
                    nc.sync.dma_start(
                        out=t3[:],
                        in_=w3t_d[h * P:(h + 1) * P, q * FQ * P:(q + 1) * FQ * P])
                    w3q.append(t3)
                w2q = []
                for fq in range(FQ):
                    f = q * FQ + fq
                    row = []
                    for hcol in range(HC):
                        t2 = w2_pool.tile([P, HW], F32, tag="w2")
                        nc.sync.dma_start(
                            out=t2[:],
                            in_=w2t_d[f * P:(f + 1) * P, hcol * HW:(hcol + 1) * HW])
                        row.append(t2)
                    w2q.append(row)

                # ---- GEMM1/2: h1T/h3T [P(F), NW] + silu*mul -> g tiles [P, TC]
                gq = []
                for fq in range(FQ):
                    p1 = [ps12.tile([P, NW], F32, tag="ps12", name=f"p1_{c}_{q}_{fq}_{th}") for th in range(TH)]
                    p3 = [ps12.tile([P, NW], F32, tag="ps12", name=f"p3_{c}_{q}_{fq}_{th}") for th in range(TH)]
                    for h in range(HT):
                        lw = w1q[h][:, fq * P:(fq + 1) * P]
                        for th in range(TH):
                            nc.tensor.matmul(
                                p1[th][:], lw, xts[h][:, th * NW:(th + 1) * NW],
                                start=(h == 0), stop=(h == HT - 1))
                    for h in range(HT):
                        lw = w3q[h][:, fq * P:(fq + 1) * P]
                        for th in range(TH):
                            nc.tensor.matmul(
                                p3[th][:], lw, xts[h][:, th * NW:(th + 1) * NW],
                                start=(h == 0), stop=(h == HT - 1))
                    gt = g_pool.tile([P, TC], F32, tag="g")
                    for th in range(TH):
                        tmp = tmp_pool.tile([P, NW], F32, tag="tmp")
                        if silu_native:
                            nc.scalar.activation(tmp[:], p1[th][:], AF.Silu)
                        else:
                            # CoreSim has no Silu; sigmoid then explicit mul
                            sg = tmp_pool.tile([P, NW], F32, tag="tmp")
                            nc.scalar.activation(sg[:], p1[th][:], AF.Sigmoid)
                            nc.vector.tensor_tensor(tmp[:], sg[:], p1[th][:], OP.mult)
                        nc.vector.tensor_tensor(
                            gt[:, th * NW:(th + 1) * NW], tmp[:], p3[th][:], OP.mult)
                    gq.append(gt)

                # ---- GEMM3: out[T-part, H-col] partial over this quarter's F
                for tt in range(NTT):
                    for hcol in range(HC):
                        po = ps3.tile([P, HW], F32, tag="ps3")
                        for fq in range(FQ):
                            nc.tensor.matmul(
                                po[:], gq[fq][:, tt * P:(tt + 1) * P], w2q[fq][hcol][:],
                                start=(fq == 0), stop=(fq == FQ - 1))
                        if q == 0:
                            at = acc_pool.tile([P, HW], F32, tag="acc")
                            acc_tiles[(tt, hcol)] = at
                            if NQ == 1:
                                ob = ob_pool.tile([P, HW], F32, tag="ob")
                                nc.vector.tensor_scalar(
                                    ob[:], po[:], scales[tt][:], None, OP.mult)
                                nc.sync.dma_start(
                                    out=out_d[t0 + tt * P:t0 + (tt + 1) * P,
                                              hcol * HW:(hcol + 1) * HW],
                                    in_=ob[:])
                            else:
                                nc.scalar.copy(at[:], po[:])
                        else:
                            at = acc_tiles[(tt, hcol)]
                            nc.vector.tensor_tensor(at[:], po[:], at[:], OP.add)
                            if q == NQ - 1:
                                ob = ob_pool.tile([P, HW], F32, tag="ob")
                                nc.vector.tensor_scalar(
                                    ob[:], at[:], scales[tt][:], None, OP.mult)
                                nc.sync.dma_start(
                                    out=out_d[t0 + tt * P:t0 + (tt + 1) * P,
                                              hcol * HW:(hcol + 1) * HW],
                                    in_=ob[:])

    nc.compile()
    return nc


def make_in_maps(hidden_states, gate_w, w1, w2, w3):
    """Shard/transpose FULL inputs into per-core in_maps (expert-parallel)."""
    B, S, H = hidden_states.shape
    E = gate_w.shape[0]
    x2 = np.asarray(hidden_states, dtype=np.float32).reshape(-1, H)
    xt = np.ascontiguousarray(x2.T)
    gt = np.asarray(gate_w, dtype=np.float32).T  # [H, E]
    in_maps = []
    for e in range(E):
        perm = [(e + j) % E for j in range(E)]
        in_maps.append({
            "xt": xt,
            "gwt": np.ascontiguousarray(gt[:, perm]),
            "w1t": np.ascontiguousarray(np.asarray(w1[e], dtype=np.float32).T),
            "w3t": np.ascontiguousarray(np.asarray(w3[e], dtype=np.float32).T),
            "w2t": np.ascontiguousarray(np.asarray(w2[e], dtype=np.float32).T),
        })
    return in_maps


_NC_CACHE = {}


def _get_nc():
    if "nc" not in _NC_CACHE:
        _NC_CACHE["nc"] = build_moe_nc()
    return _NC_CACHE["nc"]


def kernel(hidden_states, gate_w, w1, w2, w3, _trace=False, _trace_kwargs=None):
    B, S, H = hidden_states.shape
    E = gate_w.shape[0]
    nc = _get_nc()
    in_maps = make_in_maps(hidden_states, gate_w, w1, w2, w3)
    res = run_bass_kernel_spmd(
        nc, in_maps, list(range(E)), trace=_trace, **(_trace_kwargs or {}))
    kernel.last_results = res
    out = np.zeros((B * S, H), dtype=np.float32)
    for r in res.results:
        out += r["out"]
    return out.reshape(B, S, H).astype(hidden_states.dtype)


# revision 6
# speedup vs baseline: 3.6012x; 3.6012x over previous
"""Mixtral MoE layer (top-2 of 8 experts) as a Trainium2 Bass/Tile kernel.

Strategy (expert-parallel, per the original tp_size/expert_indices code):
  - 8 NeuronCores, one expert per core. Gate is replicated.
  - Host pre-transposes x -> xT [H,T] and weights -> w1t/w3t [H,F], w2t [F,H]
    so every SBUF tile load is a natural contiguous 2D slice (no on-device
    transposes anywhere).
  - On device, each core:
      * computes router logits for all tokens (xT-tile as stationary operand,
        gate.T as moving), softmax + top-2 + renorm along the free dim; the
        softmax denominator cancels in the top-2 renormalization, so the
        per-token weight for expert e is exp_e * [exp_e >= v2] / (v1 + v2).
      * runs its expert's MLP densely over all tokens in transposed space:
        h1T/h3T [F-part, T-col] = w1t/w3t-tile.T @ xT, g = silu(h1T)*h3T,
        out [T-part, H-col] = gT-tile.T @ w2t  (g is already in the exact
        stationary layout GEMM3 wants).
      * scales out rows by its expert's per-token routing weight (a
        per-partition scalar in this layout) and writes a [T,H] partial.
  - Host sums the 8 partials (the "all-reduce") and reshapes to [B,S,H].
  - Per-core column-0 trick: each core's gate copy has its columns permuted
    so that column 0 is its own expert -> one SPMD program, no dynamic
    indexing.

Tiling: tokens in chunks of TC=1024; F processed in NQ=4 quarters with an
SBUF fp32 accumulator for the GEMM3 partial sums so that the g-chunk and the
w2 tiles both stay small enough for SBUF.
"""

from contextlib import ExitStack

import ml_dtypes
import numpy as np

import concourse.bacc as bacc
import concourse.tile as tile
from concourse import mybir
from concourse.bass_utils import run_bass_kernel_spmd

P = 128
AF = mybir.ActivationFunctionType
OP = mybir.AluOpType
AX = mybir.AxisListType
F32 = mybir.dt.float32


def build_moe_nc(T=4096, H=1024, F=3584, E=8, TC=1024, NQ=4, silu_native=True,
                 mm_bf16=True):
    """Build the single-core SPMD program. Returns the compiled Bacc."""
    HT = H // P          # contraction k-tiles for GEMM1/2
    FT = F // P          # f tiles
    FQ = FT // NQ        # f-tiles per quarter
    NCH = T // TC        # token chunks
    NW = min(512, TC)    # moving free dim per matmul (fp32 max 512)
    TH = TC // NW        # moving-operand slices per chunk
    NTT = TC // P        # 128-token tiles per chunk
    HW = min(512, H)     # output H column group width
    HC = H // HW
    assert FT % NQ == 0 and T % TC == 0 and TC % P == 0 and H % HW == 0

    MDT = mybir.dt.bfloat16 if mm_bf16 else F32
    nc = bacc.Bacc("TRN2", target_bir_lowering=False, debug=False)
    xt_d = nc.dram_tensor("xt", [H, T], F32, kind="ExternalInput").ap()
    gwt_d = nc.dram_tensor("gwt", [H, E], F32, kind="ExternalInput").ap()
    w1t_d = nc.dram_tensor("w1t", [H, F], MDT, kind="ExternalInput").ap()
    w3t_d = nc.dram_tensor("w3t", [H, F], MDT, kind="ExternalInput").ap()
    w2t_d = nc.dram_tensor("w2t", [F, H], MDT, kind="ExternalInput").ap()
    out_d = nc.dram_tensor("out", [T, H], F32, kind="ExternalOutput").ap()

    with tile.TileContext(nc) as tc, ExitStack() as ctx:
        xt_pool = ctx.enter_context(tc.tile_pool(name="xt", bufs=HT))
        xtb_pool = ctx.enter_context(tc.tile_pool(name="xtb", bufs=HT))
        gw_pool = ctx.enter_context(tc.tile_pool(name="gw", bufs=HT))
        w13_pool = ctx.enter_context(tc.tile_pool(name="w13", bufs=2 * HT))
        w2_pool = ctx.enter_context(tc.tile_pool(name="w2", bufs=FQ * HC))
        g_pool = ctx.enter_context(tc.tile_pool(name="g", bufs=FQ))
        acc_pool = ctx.enter_context(tc.tile_pool(name="acc", bufs=NTT * HC))
        tmp_pool = ctx.enter_context(tc.tile_pool(name="tmp", bufs=4))
        rt_pool = ctx.enter_context(tc.tile_pool(name="rt", bufs=16))
        sc_pool = ctx.enter_context(tc.tile_pool(name="sc", bufs=2 * NTT))
        ob_pool = ctx.enter_context(tc.tile_pool(name="ob", bufs=4))
        ps12 = ctx.enter_context(tc.tile_pool(name="ps12", bufs=6, space="PSUM"))
        ps3 = ctx.enter_context(tc.tile_pool(name="ps3", bufs=2, space="PSUM"))

        # gate weights resident for the whole kernel
        gw_tiles = []
        for h in range(HT):
            t_ = gw_pool.tile([P, E], F32, tag="gw")
            nc.sync.dma_start(out=t_[:], in_=gwt_d[h * P:(h + 1) * P, :])
            gw_tiles.append(t_)

        for c in range(NCH):
            t0 = c * TC
            # ---- xT chunk [H, TC] as HT tiles of [P, TC]
            xts = []
            xtb = []
            for h in range(HT):
                t_ = xt_pool.tile([P, TC], F32, tag="xt")
                nc.sync.dma_start(out=t_[:], in_=xt_d[h * P:(h + 1) * P, t0:t0 + TC])
                xts.append(t_)
                if mm_bf16:
                    tb = xtb_pool.tile([P, TC], MDT, tag="xtb")
                    nc.gpsimd.tensor_copy(tb[:], t_[:])
                    xtb.append(tb)
                else:
                    xtb.append(t_)

            # ---- routing for this chunk: per 128-token tile
            scales = []
            for tt in range(NTT):
                psr = ps3.tile([P, E], F32, tag="ps3")
                for h in range(HT):
                    nc.tensor.matmul(
                        psr[:], xts[h][:, tt * P:(tt + 1) * P], gw_tiles[h][:],
                        start=(h == 0), stop=(h == HT - 1))
                lg = rt_pool.tile([P, E], F32, tag="rt")
                nc.scalar.copy(lg[:], psr[:])
                nm = rt_pool.tile([P, 1], F32, tag="rt")
                nc.vector.tensor_reduce(nm[:], lg[:], AX.X, OP.max, negate=True)
                ex = rt_pool.tile([P, E], F32, tag="rt")
                nc.scalar.activation(ex[:], lg[:], AF.Exp, bias=nm[:])
                v1 = rt_pool.tile([P, 1], F32, tag="rt")
                nc.vector.tensor_reduce(v1[:], ex[:], AX.X, OP.max)
                ltm = rt_pool.tile([P, E], F32, tag="rt")
                nc.vector.tensor_scalar(ltm[:], ex[:], v1[:], None, OP.is_lt)
                e2 = rt_pool.tile([P, E], F32, tag="rt")
                nc.vector.tensor_tensor(e2[:], ex[:], ltm[:], OP.mult)
                v2 = rt_pool.tile([P, 1], F32, tag="rt")
                nc.vector.tensor_reduce(v2[:], e2[:], AX.X, OP.max)
                den = rt_pool.tile([P, 1], F32, tag="rt")
                nc.vector.tensor_tensor(den[:], v1[:], v2[:], OP.add)
                rd = rt_pool.tile([P, 1], F32, tag="rt")
                nc.vector.reciprocal(rd[:], den[:])
                # column 0 is this core's expert; weight = ex0*[ex0>=v2]/(v1+v2)
                ge = rt_pool.tile([P, 1], F32, tag="rt")
                nc.vector.tensor_scalar(ge[:], ex[:, 0:1], v2[:], None, OP.is_ge)
                w0 = rt_pool.tile([P, 1], F32, tag="rt")
                nc.vector.tensor_tensor(w0[:], ex[:, 0:1], ge[:], OP.mult)
                sc = sc_pool.tile([P, 1], F32, tag="sc")
                nc.vector.tensor_tensor(sc[:], w0[:], rd[:], OP.mult)
                scales.append(sc)

            acc_tiles = {}
            for q in range(NQ):
                # ---- quarter weight loads: w1/w3 [P, FQ*P] per h-tile
                w1q, w3q = [], []
                for h in range(HT):
                    t1 = w13_pool.tile([P, FQ * P], MDT, tag="w13")
                    nc.sync.dma_start(
                        out=t1[:],
                        in_=w1t_d[h * P:(h + 1) * P, q * FQ * P:(q + 1) * FQ * P])
                    w1q.append(t1)
                    t3 = w13_pool.tile([P, FQ * P], MDT, tag="w13")
                    nc.sync.dma_start(
                        out=t3[:],
                        in_=w3t_d[h * P:(h + 1) * P, q * FQ * P:(q + 1) * FQ * P])
                    w3q.append(t3)
                w2q = []
                for fq in range(FQ):
                    f = q * FQ + fq
                    row = []
                    for hcol in range(HC):
                        t2 = w2_pool.tile([P, HW], MDT, tag="w2")
                        nc.sync.dma_start(
                            out=t2[:],
                            in_=w2t_d[f * P:(f + 1) * P, hcol * HW:(hcol + 1) * HW])
                        row.append(t2)
                    w2q.append(row)

                # ---- GEMM1/2: h1T/h3T [P(F), NW] + silu*mul -> g tiles [P, TC]
                gq = []
                for fq in range(FQ):
                    p1 = [ps12.tile([P, NW], F32, tag="ps12", name=f"p1_{c}_{q}_{fq}_{th}") for th in range(TH)]
                    p3 = [ps12.tile([P, NW], F32, tag="ps12", name=f"p3_{c}_{q}_{fq}_{th}") for th in range(TH)]
                    for h in range(HT):
                        lw = w1q[h][:, fq * P:(fq + 1) * P]
                        for th in range(TH):
                            nc.tensor.matmul(
                                p1[th][:], lw, xtb[h][:, th * NW:(th + 1) * NW],
                                start=(h == 0), stop=(h == HT - 1))
                    for h in range(HT):
                        lw = w3q[h][:, fq * P:(fq + 1) * P]
                        for th in range(TH):
                            nc.tensor.matmul(
                                p3[th][:], lw, xtb[h][:, th * NW:(th + 1) * NW],
                                start=(h == 0), stop=(h == HT - 1))
                    gt = g_pool.tile([P, TC], MDT, tag="g")
                    for th in range(TH):
                        tmp = tmp_pool.tile([P, NW], F32, tag="tmp")
                        if silu_native:
                            nc.scalar.activation(tmp[:], p1[th][:], AF.Silu)
                        else:
                            # CoreSim has no Silu; sigmoid then explicit mul
                            sg = tmp_pool.tile([P, NW], F32, tag="tmp")
                            nc.scalar.activation(sg[:], p1[th][:], AF.Sigmoid)
                            nc.vector.tensor_tensor(tmp[:], sg[:], p1[th][:], OP.mult)
                        nc.vector.tensor_tensor(
                            gt[:, th * NW:(th + 1) * NW], tmp[:], p3[th][:], OP.mult)
                    gq.append(gt)

                # ---- GEMM3: out[T-part, H-col] partial over this quarter's F
                for tt in range(NTT):
                    for hcol in range(HC):
                        po = ps3.tile([P, HW], F32, tag="ps3")
                        for fq in range(FQ):
                            nc.tensor.matmul(
                                po[:], gq[fq][:, tt * P:(tt + 1) * P], w2q[fq][hcol][:],
                                start=(fq == 0), stop=(fq == FQ - 1))
                        if q == 0:
                            at = acc_pool.tile([P, HW], F32, tag="acc")
                            acc_tiles[(tt, hcol)] = at
                            if NQ == 1:
                                ob = ob_pool.tile([P, HW], F32, tag="ob")
                                nc.vector.tensor_scalar(
                                    ob[:], po[:], scales[tt][:], None, OP.mult)
                                nc.sync.dma_start(
                                    out=out_d[t0 + tt * P:t0 + (tt + 1) * P,
                                              hcol * HW:(hcol + 1) * HW],
                                    in_=ob[:])
                            else:
                                nc.scalar.copy(at[:], po[:])
                        else:
                            at = acc_tiles[(tt, hcol)]
                            nc.vector.tensor_tensor(at[:], po[:], at[:], OP.add)
                            if q == NQ - 1:
                                ob = ob_pool.tile([P, HW], F32, tag="ob")
                                nc.vector.tensor_scalar(
                                    ob[:], at[:], scales[tt][:], None, OP.mult)
                                nc.sync.dma_start(
                                    out=out_d[t0 + tt * P:t0 + (tt + 1) * P,
                                              hcol * HW:(hcol + 1) * HW],
                                    in_=ob[:])

    nc.compile()
    return nc


def make_in_maps(hidden_states, gate_w, w1, w2, w3, mm_bf16=True):
    """Shard/transpose FULL inputs into per-core in_maps (expert-parallel)."""
    B, S, H = hidden_states.shape
    E = gate_w.shape[0]
    wdt = ml_dtypes.bfloat16 if mm_bf16 else np.float32
    x2 = np.asarray(hidden_states, dtype=np.float32).reshape(-1, H)
    xt = np.ascontiguousarray(x2.T)
    gt = np.asarray(gate_w, dtype=np.float32).T  # [H, E]
    in_maps = []
    for e in range(E):
        perm = [(e + j) % E for j in range(E)]
        in_maps.append({
            "xt": xt,
            "gwt": np.ascontiguousarray(gt[:, perm]),
            "w1t": np.ascontiguousarray(np.asarray(w1[e], dtype=np.float32).T).astype(wdt),
            "w3t": np.ascontiguousarray(np.asarray(w3[e], dtype=np.float32).T).astype(wdt),
            "w2t": np.ascontiguousarray(np.asarray(w2[e], dtype=np.float32).T).astype(wdt),
        })
    return in_maps


_NC_CACHE = {}


def _get_nc():
    if "nc" not in _NC_CACHE:
        _NC_CACHE["nc"] = build_moe_nc()
    return _NC_CACHE["nc"]


def kernel(hidden_states, gate_w, w1, w2, w3, _trace=False, _trace_kwargs=None):
    B, S, H = hidden_states.shape
    E = gate_w.shape[0]
    nc = _get_nc()
    in_maps = make_in_maps(hidden_states, gate_w, w1, w2, w3)
    res = run_bass_kernel_spmd(
        nc, in_maps, list(range(E)), trace=_trace, **(_trace_kwargs or {}))
    kernel.last_results = res
    out = np.zeros((B * S, H), dtype=np.float32)
    for r in res.results:
        out += r["out"]
    return out.reshape(B, S, H).astype(hidden_states.dtype)


# revision 8
# speedup vs baseline: 11.8789x; 3.2986x over previous
"""Mixtral MoE layer (top-2 of 8 experts) as a Trainium2 Bass/Tile kernel.

Strategy (expert-parallel, per the original tp_size/expert_indices code):
  - 8 NeuronCores, one expert per core. Gate is replicated.
  - Host pre-transposes x -> xT [H,T] and weights -> w1t/w3t [H,F], w2t [F,H]
    so every SBUF tile load is a natural contiguous 2D slice (no on-device
    transposes anywhere).
  - On device, each core:
      * computes router logits for all tokens (xT-tile as stationary operand,
        gate.T as moving), softmax + top-2 + renorm along the free dim; the
        softmax denominator cancels in the top-2 renormalization, so the
        per-token weight for expert e is exp_e * [exp_e >= v2] / (v1 + v2).
      * runs its expert's MLP densely over all tokens in transposed space:
        h1T/h3T [F-part, T-col] = w1t/w3t-tile.T @ xT, g = silu(h1T)*h3T,
        out [T-part, H-col] = gT-tile.T @ w2t  (g is already in the exact
        stationary layout GEMM3 wants).
      * scales out rows by its expert's per-token routing weight (a
        per-partition scalar in this layout) and writes a [T,H] partial.
  - Host sums the 8 partials (the "all-reduce") and reshapes to [B,S,H].
  - Per-core column-0 trick: each core's gate copy has its columns permuted
    so that column 0 is its own expert -> one SPMD program, no dynamic
    indexing.

Tiling: tokens in chunks of TC=1024; F processed in NQ=4 quarters with an
SBUF fp32 accumulator for the GEMM3 partial sums so that the g-chunk and the
w2 tiles both stay small enough for SBUF.
"""

from contextlib import ExitStack

import ml_dtypes
import numpy as np

import concourse.bacc as bacc
import concourse.tile as tile
from concourse import mybir
from concourse.bass_utils import run_bass_kernel_spmd

P = 128
AF = mybir.ActivationFunctionType
OP = mybir.AluOpType
AX = mybir.AxisListType
F32 = mybir.dt.float32


def build_moe_nc(T=4096, H=1024, F=3584, E=8, TC=1024, NQ=4, silu_native=True,
                 mm_bf16=True):
    """Build the single-core SPMD program. Returns the compiled Bacc."""
    HT = H // P          # contraction k-tiles for GEMM1/2
    FT = F // P          # f tiles
    FQ = FT // NQ        # f-tiles per quarter
    NCH = T // TC        # token chunks
    # moving-operand slices of the token chunk (fp32/bf16 max free dim 512)
    nw_slices = []
    off = 0
    while off < TC:
        w = min(512, TC - off)
        nw_slices.append((off, w))
        off += w
    TH = len(nw_slices)
    NTT = TC // P        # 128-token tiles per chunk
    HW = min(512, H)     # output H column group width
    HC = H // HW
    assert FT % NQ == 0 and T % TC == 0 and TC % P == 0 and H % HW == 0

    MDT = mybir.dt.bfloat16 if mm_bf16 else F32
    nc = bacc.Bacc("TRN2", target_bir_lowering=False, debug=False)
    xt_d = nc.dram_tensor("xt", [H, T], F32, kind="ExternalInput").ap()
    gwt_d = nc.dram_tensor("gwt", [H, E], F32, kind="ExternalInput").ap()
    w1t_d = nc.dram_tensor("w1t", [H, F], MDT, kind="ExternalInput").ap()
    w3t_d = nc.dram_tensor("w3t", [H, F], MDT, kind="ExternalInput").ap()
    w2t_d = nc.dram_tensor("w2t", [F, H], MDT, kind="ExternalInput").ap()
    out_d = nc.dram_tensor("out", [T, H], F32, kind="ExternalOutput").ap()

    with tile.TileContext(nc) as tc, ExitStack() as ctx:
        xt_pool = ctx.enter_context(tc.tile_pool(name="xt", bufs=HT))
        xtb_pool = ctx.enter_context(tc.tile_pool(name="xtb", bufs=HT))
        gw_pool = ctx.enter_context(tc.tile_pool(name="gw", bufs=HT))
        w13_pool = ctx.enter_context(tc.tile_pool(name="w13", bufs=2 * HT))
        w2_pool = ctx.enter_context(tc.tile_pool(name="w2", bufs=FQ * HC))
        g_pool = ctx.enter_context(tc.tile_pool(name="g", bufs=FQ))
        acc_pool = ctx.enter_context(tc.tile_pool(name="acc", bufs=NTT * HC))
        tmp_pool = ctx.enter_context(tc.tile_pool(name="tmp", bufs=4))
        rt_pool = ctx.enter_context(tc.tile_pool(name="rt", bufs=16))
        sc_pool = ctx.enter_context(tc.tile_pool(name="sc", bufs=2 * NTT))
        ob_pool = ctx.enter_context(tc.tile_pool(name="ob", bufs=4))
        ps12 = ctx.enter_context(tc.tile_pool(name="ps12", bufs=6, space="PSUM"))
        ps3 = ctx.enter_context(tc.tile_pool(name="ps3", bufs=2, space="PSUM"))

        # gate weights resident for the whole kernel
        gw_tiles = []
        for h in range(HT):
            t_ = gw_pool.tile([P, E], F32, tag="gw")
            nc.sync.dma_start(out=t_[:], in_=gwt_d[h * P:(h + 1) * P, :])
            gw_tiles.append(t_)

        for c in range(NCH):
            t0 = c * TC
            # ---- xT chunk [H, TC] as HT tiles of [P, TC]
            xts = []
            xtb = []
            for h in range(HT):
                t_ = xt_pool.tile([P, TC], F32, tag="xt")
                nc.sync.dma_start(out=t_[:], in_=xt_d[h * P:(h + 1) * P, t0:t0 + TC])
                xts.append(t_)
                if mm_bf16:
                    tb = xtb_pool.tile([P, TC], MDT, tag="xtb")
                    nc.gpsimd.tensor_copy(tb[:], t_[:])
                    xtb.append(tb)
                else:
                    xtb.append(t_)

            # ---- routing for this chunk: per 128-token tile
            scales = []
            for tt in range(NTT):
                psr = ps3.tile([P, E], F32, tag="ps3")
                for h in range(HT):
                    nc.tensor.matmul(
                        psr[:], xts[h][:, tt * P:(tt + 1) * P], gw_tiles[h][:],
                        start=(h == 0), stop=(h == HT - 1))
                lg = rt_pool.tile([P, E], F32, tag="rt")
                nc.scalar.copy(lg[:], psr[:])
                nm = rt_pool.tile([P, 1], F32, tag="rt")
                nc.vector.tensor_reduce(nm[:], lg[:], AX.X, OP.max, negate=True)
                ex = rt_pool.tile([P, E], F32, tag="rt")
                nc.scalar.activation(ex[:], lg[:], AF.Exp, bias=nm[:])
                v1 = rt_pool.tile([P, 1], F32, tag="rt")
                nc.vector.tensor_reduce(v1[:], ex[:], AX.X, OP.max)
                ltm = rt_pool.tile([P, E], F32, tag="rt")
                nc.vector.tensor_scalar(ltm[:], ex[:], v1[:], None, OP.is_lt)
                e2 = rt_pool.tile([P, E], F32, tag="rt")
                nc.vector.tensor_tensor(e2[:], ex[:], ltm[:], OP.mult)
                v2 = rt_pool.tile([P, 1], F32, tag="rt")
                nc.vector.tensor_reduce(v2[:], e2[:], AX.X, OP.max)
                den = rt_pool.tile([P, 1], F32, tag="rt")
                nc.vector.tensor_tensor(den[:], v1[:], v2[:], OP.add)
                rd = rt_pool.tile([P, 1], F32, tag="rt")
                nc.vector.reciprocal(rd[:], den[:])
                # column 0 is this core's expert; weight = ex0*[ex0>=v2]/(v1+v2)
                ge = rt_pool.tile([P, 1], F32, tag="rt")
                nc.vector.tensor_scalar(ge[:], ex[:, 0:1], v2[:], None, OP.is_ge)
                w0 = rt_pool.tile([P, 1], F32, tag="rt")
                nc.vector.tensor_tensor(w0[:], ex[:, 0:1], ge[:], OP.mult)
                sc = sc_pool.tile([P, 1], F32, tag="sc")
                nc.vector.tensor_tensor(sc[:], w0[:], rd[:], OP.mult)
                scales.append(sc)

            acc_tiles = {}
            for q in range(NQ):
                # ---- quarter weight loads: w1/w3 [P, FQ*P] per h-tile
                w1q, w3q = [], []
                for h in range(HT):
                    t1 = w13_pool.tile([P, FQ * P], MDT, tag="w13")
                    nc.sync.dma_start(
                        out=t1[:],
                        in_=w1t_d[h * P:(h + 1) * P, q * FQ * P:(q + 1) * FQ * P])
                    w1q.append(t1)
                    t3 = w13_pool.tile([P, FQ * P], MDT, tag="w13")
                    nc.sync.dma_start(
                        out=t3[:],
                        in_=w3t_d[h * P:(h + 1) * P, q * FQ * P:(q + 1) * FQ * P])
                    w3q.append(t3)
                w2q = []
                for fq in range(FQ):
                    f = q * FQ + fq
                    row = []
                    for hcol in range(HC):
                        t2 = w2_pool.tile([P, HW], MDT, tag="w2")
                        nc.sync.dma_start(
                            out=t2[:],
                            in_=w2t_d[f * P:(f + 1) * P, hcol * HW:(hcol + 1) * HW])
                        row.append(t2)
                    w2q.append(row)

                # ---- GEMM1/2: h1T/h3T [P(F), NW] + silu*mul -> g tiles [P, TC]
                gq = []
                for fq in range(FQ):
                    p1 = [ps12.tile([P, w], F32, tag="ps12", name=f"p1_{c}_{q}_{fq}_{th}")
                          for th, (o, w) in enumerate(nw_slices)]
                    p3 = [ps12.tile([P, w], F32, tag="ps12", name=f"p3_{c}_{q}_{fq}_{th}")
                          for th, (o, w) in enumerate(nw_slices)]
                    for h in range(HT):
                        lw = w1q[h][:, fq * P:(fq + 1) * P]
                        for th, (o, w) in enumerate(nw_slices):
                            nc.tensor.matmul(
                                p1[th][:], lw, xtb[h][:, o:o + w],
                                start=(h == 0), stop=(h == HT - 1))
                    for h in range(HT):
                        lw = w3q[h][:, fq * P:(fq + 1) * P]
                        for th, (o, w) in enumerate(nw_slices):
                            nc.tensor.matmul(
                                p3[th][:], lw, xtb[h][:, o:o + w],
                                start=(h == 0), stop=(h == HT - 1))
                    gt = g_pool.tile([P, TC], MDT, tag="g")
                    for th, (o, w) in enumerate(nw_slices):
                        tmp = tmp_pool.tile([P, w], F32, tag="tmp")
                        if silu_native:
                            nc.scalar.activation(tmp[:], p1[th][:], AF.Silu)
                        else:
                            # CoreSim has no Silu; sigmoid then explicit mul
                            sg = tmp_pool.tile([P, w], F32, tag="tmp")
                            nc.scalar.activation(sg[:], p1[th][:], AF.Sigmoid)
                            nc.vector.tensor_tensor(tmp[:], sg[:], p1[th][:], OP.mult)
                        nc.vector.tensor_tensor(
                            gt[:, o:o + w], tmp[:], p3[th][:], OP.mult)
                    gq.append(gt)

                # ---- GEMM3: out[T-part, H-col] partial over this quarter's F
                for tt in range(NTT):
                    for hcol in range(HC):
                        po = ps3.tile([P, HW], F32, tag="ps3")
                        for fq in range(FQ):
                            nc.tensor.matmul(
                                po[:], gq[fq][:, tt * P:(tt + 1) * P], w2q[fq][hcol][:],
                                start=(fq == 0), stop=(fq == FQ - 1))
                        if q == 0:
                            at = acc_pool.tile([P, HW], F32, tag="acc")
                            acc_tiles[(tt, hcol)] = at
                            if NQ == 1:
                                ob = ob_pool.tile([P, HW], F32, tag="ob")
                                nc.vector.tensor_scalar(
                                    ob[:], po[:], scales[tt][:], None, OP.mult)
                                nc.sync.dma_start(
                                    out=out_d[t0 + tt * P:t0 + (tt + 1) * P,
                                              hcol * HW:(hcol + 1) * HW],
                                    in_=ob[:])
                            else:
                                nc.scalar.copy(at[:], po[:])
                        else:
                            at = acc_tiles[(tt, hcol)]
                            nc.vector.tensor_tensor(at[:], po[:], at[:], OP.add)
                            if q == NQ - 1:
                                ob = ob_pool.tile([P, HW], F32, tag="ob")
                                nc.vector.tensor_scalar(
                                    ob[:], at[:], scales[tt][:], None, OP.mult)
                                nc.sync.dma_start(
                                    out=out_d[t0 + tt * P:t0 + (tt + 1) * P,
                                              hcol * HW:(hcol + 1) * HW],
                                    in_=ob[:])

    nc.compile()
    return nc


def make_in_maps(hidden_states, gate_w, w1, w2, w3, mm_bf16=True):
    """Shard/transpose FULL inputs into per-core in_maps (expert-parallel)."""
    B, S, H = hidden_states.shape
    E = gate_w.shape[0]
    wdt = ml_dtypes.bfloat16 if mm_bf16 else np.float32
    x2 = np.asarray(hidden_states, dtype=np.float32).reshape(-1, H)
    xt = np.ascontiguousarray(x2.T)
    gt = np.asarray(gate_w, dtype=np.float32).T  # [H, E]
    in_maps = []
    for e in range(E):
        perm = [(e + j) % E for j in range(E)]
        in_maps.append({
            "xt": xt,
            "gwt": np.ascontiguousarray(gt[:, perm]),
            "w1t": np.ascontiguousarray(np.asarray(w1[e], dtype=np.float32).T).astype(wdt),
            "w3t": np.ascontiguousarray(np.asarray(w3[e], dtype=np.float32).T).astype(wdt),
            "w2t": np.ascontiguousarray(np.asarray(w2[e], dtype=np.float32).T).astype(wdt),
        })
    return in_maps


_NC_CACHE = {}


def _get_nc(key, **kw):
    if key not in _NC_CACHE:
        _NC_CACHE[key] = build_moe_nc(**kw)
    return _NC_CACHE[key]


def _host_top2_idx(x2, gate_w):
    """Token index list per expert (host copy of the routing, for sharding).

    The device recomputes the routing weights itself; this only decides
    which (token, expert) pairs each core works on.
    """
    logits = x2.astype(np.float32) @ gate_w.astype(np.float32).T
    order = np.argsort(-logits, axis=1, kind="stable")[:, :2]
    E = gate_w.shape[0]
    return [np.nonzero((order == e).any(axis=1))[0] for e in range(E)]


def kernel(hidden_states, gate_w, w1, w2, w3, _trace=False, _trace_kwargs=None):
    B, S, H = hidden_states.shape
    E = gate_w.shape[0]
    T = B * S
    x2 = np.asarray(hidden_states, dtype=np.float32).reshape(T, H)
    idx = _host_top2_idx(x2, gate_w)
    cmax = max(len(i) for i in idx)
    cpad = max(512, -(-cmax // P) * P)

    if cpad <= 2048:
        # sparse path: each core gets only its expert's tokens (padded)
        nc = _get_nc(("sparse", cpad), T=cpad, TC=cpad, NQ=4)
        xt = np.ascontiguousarray(x2.T)
        base = make_in_maps(hidden_states, gate_w, w1, w2, w3)
        in_maps = []
        for e in range(E):
            xg = np.zeros((H, cpad), dtype=np.float32)
            xg[:, :len(idx[e])] = xt[:, idx[e]]
            m = dict(base[e])
            m["xt"] = xg
            in_maps.append(m)
        res = run_bass_kernel_spmd(
            nc, in_maps, list(range(E)), trace=_trace, **(_trace_kwargs or {}))
        kernel.last_results = res
        out = np.zeros((T, H), dtype=np.float32)
        for e, r in enumerate(res.results):
            out[idx[e]] += r["out"][:len(idx[e])]
    else:
        # dense fallback (pathological routing imbalance)
        nc = _get_nc(("dense",), T=T, TC=1024, NQ=4)
        in_maps = make_in_maps(hidden_states, gate_w, w1, w2, w3)
        res = run_bass_kernel_spmd(
            nc, in_maps, list(range(E)), trace=_trace, **(_trace_kwargs or {}))
        kernel.last_results = res
        out = np.zeros((T, H), dtype=np.float32)
        for r in res.results:
            out += r["out"]
    return out.reshape(B, S, H).astype(hidden_states.dtype)


# revision 9
# speedup vs baseline: 12.0298x; 1.0127x over previous
"""Mixtral MoE layer (top-2 of 8 experts) as a Trainium2 Bass/Tile kernel.

Strategy (expert-parallel, per the original tp_size/expert_indices code):
  - 8 NeuronCores, one expert per core. Gate is replicated.
  - Host pre-transposes x -> xT [H,T] and weights -> w1t/w3t [H,F], w2t [F,H]
    so every SBUF tile load is a natural contiguous 2D slice (no on-device
    transposes anywhere).
  - On device, each core:
      * computes router logits for all tokens (xT-tile as stationary operand,
        gate.T as moving), softmax + top-2 + renorm along the free dim; the
        softmax denominator cancels in the top-2 renormalization, so the
        per-token weight for expert e is exp_e * [exp_e >= v2] / (v1 + v2).
      * runs its expert's MLP densely over all tokens in transposed space:
        h1T/h3T [F-part, T-col] = w1t/w3t-tile.T @ xT, g = silu(h1T)*h3T,
        out [T-part, H-col] = gT-tile.T @ w2t  (g is already in the exact
        stationary layout GEMM3 wants).
      * scales out rows by its expert's per-token routing weight (a
        per-partition scalar in this layout) and writes a [T,H] partial.
  - Host sums the 8 partials (the "all-reduce") and reshapes to [B,S,H].
  - Per-core column-0 trick: each core's gate copy has its columns permuted
    so that column 0 is its own expert -> one SPMD program, no dynamic
    indexing.

Tiling: tokens in chunks of TC=1024; F processed in NQ=4 quarters with an
SBUF fp32 accumulator for the GEMM3 partial sums so that the g-chunk and the
w2 tiles both stay small enough for SBUF.
"""

from contextlib import ExitStack

import ml_dtypes
import numpy as np

import concourse.bacc as bacc
import concourse.tile as tile
from concourse import mybir
from concourse.bass_utils import run_bass_kernel_spmd

P = 128
AF = mybir.ActivationFunctionType
OP = mybir.AluOpType
AX = mybir.AxisListType
F32 = mybir.dt.float32


def build_moe_nc(T=4096, H=1024, F=3584, E=8, TC=1024, NQ=4, silu_native=True,
                 mm_bf16=True):
    """Build the single-core SPMD program. Returns the compiled Bacc."""
    HT = H // P          # contraction k-tiles for GEMM1/2
    FT = F // P          # f tiles
    FQ = FT // NQ        # f-tiles per quarter
    NCH = T // TC        # token chunks
    # moving-operand slices of the token chunk (fp32/bf16 max free dim 512)
    nw_slices = []
    off = 0
    while off < TC:
        w = min(512, TC - off)
        nw_slices.append((off, w))
        off += w
    TH = len(nw_slices)
    NTT = TC // P        # 128-token tiles per chunk
    HW = min(512, H)     # output H column group width
    HC = H // HW
    assert FT % NQ == 0 and T % TC == 0 and TC % P == 0 and H % HW == 0

    MDT = mybir.dt.bfloat16 if mm_bf16 else F32
    nc = bacc.Bacc("TRN2", target_bir_lowering=False, debug=False)
    xt_d = nc.dram_tensor("xt", [H, T], F32, kind="ExternalInput").ap()
    gwt_d = nc.dram_tensor("gwt", [H, E], F32, kind="ExternalInput").ap()
    w1t_d = nc.dram_tensor("w1t", [H, F], MDT, kind="ExternalInput").ap()
    w3t_d = nc.dram_tensor("w3t", [H, F], MDT, kind="ExternalInput").ap()
    w2t_d = nc.dram_tensor("w2t", [F, H], MDT, kind="ExternalInput").ap()
    out_d = nc.dram_tensor("out", [T, H], F32, kind="ExternalOutput").ap()

    with tile.TileContext(nc) as tc, ExitStack() as ctx:
        xt_pool = ctx.enter_context(tc.tile_pool(name="xt", bufs=HT))
        xtb_pool = ctx.enter_context(tc.tile_pool(name="xtb", bufs=HT))
        gw_pool = ctx.enter_context(tc.tile_pool(name="gw", bufs=HT))
        w13_pool = ctx.enter_context(tc.tile_pool(name="w13", bufs=2 * HT))
        w2_pool = ctx.enter_context(tc.tile_pool(name="w2", bufs=FQ * HC))
        g_pool = ctx.enter_context(tc.tile_pool(name="g", bufs=FQ))
        acc_pool = ctx.enter_context(tc.tile_pool(name="acc", bufs=NTT * HC))
        tmp_pool = ctx.enter_context(tc.tile_pool(name="tmp", bufs=4))
        rt_pool = ctx.enter_context(tc.tile_pool(name="rt", bufs=16))
        sc_pool = ctx.enter_context(tc.tile_pool(name="sc", bufs=2 * NTT))
        ob_pool = ctx.enter_context(tc.tile_pool(name="ob", bufs=4))
        ps12 = ctx.enter_context(tc.tile_pool(name="ps12", bufs=6, space="PSUM"))
        ps3 = ctx.enter_context(tc.tile_pool(name="ps3", bufs=2, space="PSUM"))

        # ---- PE warm-up: dense dummy matmuls while the first DMAs land.
        # The HAM clock gate releases (1.2 -> 2.4 GHz) only after a sustained
        # busy window; burn it on zeros during the initial transfer instead
        # of on the first real tiles.
        wu_w = tmp_pool.tile([P, P], MDT, tag="wu")
        wu_x = tmp_pool.tile([P, 512], MDT, tag="wu2")
        nc.vector.memset(wu_w[:], 0.0)
        nc.vector.memset(wu_x[:], 0.0)
        wu_ps = ps3.tile([P, 512], F32, tag="ps3")
        for i in range(40):
            nc.tensor.matmul(wu_ps[:], wu_w[:], wu_x[:],
                             start=(i == 0), stop=(i == 39))

        # gate weights resident for the whole kernel
        gw_tiles = []
        for h in range(HT):
            t_ = gw_pool.tile([P, E], F32, tag="gw")
            nc.sync.dma_start(out=t_[:], in_=gwt_d[h * P:(h + 1) * P, :])
            gw_tiles.append(t_)

        for c in range(NCH):
            t0 = c * TC
            # ---- xT chunk [H, TC] as HT tiles of [P, TC]
            xts = []
            xtb = []
            for h in range(HT):
                t_ = xt_pool.tile([P, TC], F32, tag="xt")
                nc.sync.dma_start(out=t_[:], in_=xt_d[h * P:(h + 1) * P, t0:t0 + TC])
                xts.append(t_)
                if mm_bf16:
                    tb = xtb_pool.tile([P, TC], MDT, tag="xtb")
                    nc.gpsimd.tensor_copy(tb[:], t_[:])
                    xtb.append(tb)
                else:
                    xtb.append(t_)

            # ---- routing for this chunk: per 128-token tile
            scales = []
            for tt in range(NTT):
                psr = ps3.tile([P, E], F32, tag="ps3")
                for h in range(HT):
                    nc.tensor.matmul(
                        psr[:], xts[h][:, tt * P:(tt + 1) * P], gw_tiles[h][:],
                        start=(h == 0), stop=(h == HT - 1))
                lg = rt_pool.tile([P, E], F32, tag="rt")
                nc.scalar.copy(lg[:], psr[:])
                nm = rt_pool.tile([P, 1], F32, tag="rt")
                nc.vector.tensor_reduce(nm[:], lg[:], AX.X, OP.max, negate=True)
                ex = rt_pool.tile([P, E], F32, tag="rt")
                nc.scalar.activation(ex[:], lg[:], AF.Exp, bias=nm[:])
                v1 = rt_pool.tile([P, 1], F32, tag="rt")
                nc.vector.tensor_reduce(v1[:], ex[:], AX.X, OP.max)
                ltm = rt_pool.tile([P, E], F32, tag="rt")
                nc.vector.tensor_scalar(ltm[:], ex[:], v1[:], None, OP.is_lt)
                e2 = rt_pool.tile([P, E], F32, tag="rt")
                nc.vector.tensor_tensor(e2[:], ex[:], ltm[:], OP.mult)
                v2 = rt_pool.tile([P, 1], F32, tag="rt")
                nc.vector.tensor_reduce(v2[:], e2[:], AX.X, OP.max)
                den = rt_pool.tile([P, 1], F32, tag="rt")
                nc.vector.tensor_tensor(den[:], v1[:], v2[:], OP.add)
                rd = rt_pool.tile([P, 1], F32, tag="rt")
                nc.vector.reciprocal(rd[:], den[:])
                # column 0 is this core's expert; weight = ex0*[ex0>=v2]/(v1+v2)
                ge = rt_pool.tile([P, 1], F32, tag="rt")
                nc.vector.tensor_scalar(ge[:], ex[:, 0:1], v2[:], None, OP.is_ge)
                w0 = rt_pool.tile([P, 1], F32, tag="rt")
                nc.vector.tensor_tensor(w0[:], ex[:, 0:1], ge[:], OP.mult)
                sc = sc_pool.tile([P, 1], F32, tag="sc")
                nc.vector.tensor_tensor(sc[:], w0[:], rd[:], OP.mult)
                scales.append(sc)

            acc_tiles = {}
            for q in range(NQ):
                # ---- quarter weight loads: w1/w3 [P, FQ*P] per h-tile
                w1q, w3q = [], []
                for h in range(HT):
                    t1 = w13_pool.tile([P, FQ * P], MDT, tag="w13")
                    nc.sync.dma_start(
                        out=t1[:],
                        in_=w1t_d[h * P:(h + 1) * P, q * FQ * P:(q + 1) * FQ * P])
                    w1q.append(t1)
                    t3 = w13_pool.tile([P, FQ * P], MDT, tag="w13")
                    nc.sync.dma_start(
                        out=t3[:],
                        in_=w3t_d[h * P:(h + 1) * P, q * FQ * P:(q + 1) * FQ * P])
                    w3q.append(t3)
                w2q = []
                for fq in range(FQ):
                    f = q * FQ + fq
                    row = []
                    for hcol in range(HC):
                        t2 = w2_pool.tile([P, HW], MDT, tag="w2")
                        nc.sync.dma_start(
                            out=t2[:],
                            in_=w2t_d[f * P:(f + 1) * P, hcol * HW:(hcol + 1) * HW])
                        row.append(t2)
                    w2q.append(row)

                # ---- GEMM1/2: h1T/h3T [P(F), NW] + silu*mul -> g tiles [P, TC]
                gq = []
                for fq in range(FQ):
                    p1 = [ps12.tile([P, w], F32, tag="ps12", name=f"p1_{c}_{q}_{fq}_{th}")
                          for th, (o, w) in enumerate(nw_slices)]
                    p3 = [ps12.tile([P, w], F32, tag="ps12", name=f"p3_{c}_{q}_{fq}_{th}")
                          for th, (o, w) in enumerate(nw_slices)]
                    for h in range(HT):
                        lw = w1q[h][:, fq * P:(fq + 1) * P]
                        for th, (o, w) in enumerate(nw_slices):
                            nc.tensor.matmul(
                                p1[th][:], lw, xtb[h][:, o:o + w],
                                start=(h == 0), stop=(h == HT - 1))
                    for h in range(HT):
                        lw = w3q[h][:, fq * P:(fq + 1) * P]
                        for th, (o, w) in enumerate(nw_slices):
                            nc.tensor.matmul(
                                p3[th][:], lw, xtb[h][:, o:o + w],
                                start=(h == 0), stop=(h == HT - 1))
                    gt = g_pool.tile([P, TC], MDT, tag="g")
                    for th, (o, w) in enumerate(nw_slices):
                        tmp = tmp_pool.tile([P, w], F32, tag="tmp")
                        if silu_native:
                            nc.scalar.activation(tmp[:], p1[th][:], AF.Silu)
                        else:
                            # CoreSim has no Silu; sigmoid then explicit mul
                            sg = tmp_pool.tile([P, w], F32, tag="tmp")
                            nc.scalar.activation(sg[:], p1[th][:], AF.Sigmoid)
                            nc.vector.tensor_tensor(tmp[:], sg[:], p1[th][:], OP.mult)
                        nc.vector.tensor_tensor(
                            gt[:, o:o + w], tmp[:], p3[th][:], OP.mult)
                    gq.append(gt)

                # ---- GEMM3: out[T-part, H-col] partial over this quarter's F
                for tt in range(NTT):
                    for hcol in range(HC):
                        po = ps3.tile([P, HW], F32, tag="ps3")
                        for fq in range(FQ):
                            nc.tensor.matmul(
                                po[:], gq[fq][:, tt * P:(tt + 1) * P], w2q[fq][hcol][:],
                                start=(fq == 0), stop=(fq == FQ - 1))
                        if q == 0:
                            at = acc_pool.tile([P, HW], F32, tag="acc")
                            acc_tiles[(tt, hcol)] = at
                            if NQ == 1:
                                ob = ob_pool.tile([P, HW], F32, tag="ob")
                                nc.vector.tensor_scalar(
                                    ob[:], po[:], scales[tt][:], None, OP.mult)
                                nc.sync.dma_start(
                                    out=out_d[t0 + tt * P:t0 + (tt + 1) * P,
                                              hcol * HW:(hcol + 1) * HW],
                                    in_=ob[:])
                            else:
                                nc.scalar.copy(at[:], po[:])
                        else:
                            at = acc_tiles[(tt, hcol)]
                            nc.vector.tensor_tensor(at[:], po[:], at[:], OP.add)
                            if q == NQ - 1:
                                ob = ob_pool.tile([P, HW], F32, tag="ob")
                                nc.vector.tensor_scalar(
                                    ob[:], at[:], scales[tt][:], None, OP.mult)
                                nc.sync.dma_start(
                                    out=out_d[t0 + tt * P:t0 + (tt + 1) * P,
                                              hcol * HW:(hcol + 1) * HW],
                                    in_=ob[:])

    nc.compile()
    return nc


def make_in_maps(hidden_states, gate_w, w1, w2, w3, mm_bf16=True):
    """Shard/transpose FULL inputs into per-core in_maps (expert-parallel)."""
    B, S, H = hidden_states.shape
    E = gate_w.shape[0]
    wdt = ml_dtypes.bfloat16 if mm_bf16 else np.float32
    x2 = np.asarray(hidden_states, dtype=np.float32).reshape(-1, H)
    xt = np.ascontiguousarray(x2.T)
    gt = np.asarray(gate_w, dtype=np.float32).T  # [H, E]
    in_maps = []
    for e in range(E):
        perm = [(e + j) % E for j in range(E)]
        in_maps.append({
            "xt": xt,
            "gwt": np.ascontiguousarray(gt[:, perm]),
            "w1t": np.ascontiguousarray(np.asarray(w1[e], dtype=np.float32).T).astype(wdt),
            "w3t": np.ascontiguousarray(np.asarray(w3[e], dtype=np.float32).T).astype(wdt),
            "w2t": np.ascontiguousarray(np.asarray(w2[e], dtype=np.float32).T).astype(wdt),
        })
    return in_maps


_NC_CACHE = {}


def _get_nc(key, **kw):
    if key not in _NC_CACHE:
        _NC_CACHE[key] = build_moe_nc(**kw)
    return _NC_CACHE[key]


def _host_top2_idx(x2, gate_w):
    """Token index list per expert (host copy of the routing, for sharding).

    The device recomputes the routing weights itself; this only decides
    which (token, expert) pairs each core works on.
    """
    logits = x2.astype(np.float32) @ gate_w.astype(np.float32).T
    order = np.argsort(-logits, axis=1, kind="stable")[:, :2]
    E = gate_w.shape[0]
    return [np.nonzero((order == e).any(axis=1))[0] for e in range(E)]


def kernel(hidden_states, gate_w, w1, w2, w3, _trace=False, _trace_kwargs=None):
    B, S, H = hidden_states.shape
    E = gate_w.shape[0]
    T = B * S
    x2 = np.asarray(hidden_states, dtype=np.float32).reshape(T, H)
    idx = _host_top2_idx(x2, gate_w)
    cmax = max(len(i) for i in idx)
    cpad = max(512, -(-cmax // P) * P)

    if cpad <= 2048:
        # sparse path: each core gets only its expert's tokens (padded)
        nc = _get_nc(("sparse", cpad), T=cpad, TC=cpad, NQ=4)
        xt = np.ascontiguousarray(x2.T)
        base = make_in_maps(hidden_states, gate_w, w1, w2, w3)
        in_maps = []
        for e in range(E):
            xg = np.zeros((H, cpad), dtype=np.float32)
            xg[:, :len(idx[e])] = xt[:, idx[e]]
            m = dict(base[e])
            m["xt"] = xg
            in_maps.append(m)
        res = run_bass_kernel_spmd(
            nc, in_maps, list(range(E)), trace=_trace, **(_trace_kwargs or {}))
        kernel.last_results = res
        out = np.zeros((T, H), dtype=np.float32)
        for e, r in enumerate(res.results):
            out[idx[e]] += r["out"][:len(idx[e])]
    else:
        # dense fallback (pathological routing imbalance)
        nc = _get_nc(("dense",), T=T, TC=1024, NQ=4)
        in_maps = make_in_maps(hidden_states, gate_w, w1, w2, w3)
        res = run_bass_kernel_spmd(
            nc, in_maps, list(range(E)), trace=_trace, **(_trace_kwargs or {}))
        kernel.last_results = res
        out = np.zeros((T, H), dtype=np.float32)
        for r in res.results:
            out += r["out"]
    return out.reshape(B, S, H).astype(hidden_states.dtype)


# revision 10
# speedup vs baseline: 12.2971x; 1.0222x over previous
"""Mixtral MoE layer (top-2 of 8 experts) as a Trainium2 Bass/Tile kernel.

Strategy (expert-parallel, per the original tp_size/expert_indices code):
  - 8 NeuronCores, one expert per core. Gate is replicated.
  - Host pre-transposes x -> xT [H,T] and weights -> w1t/w3t [H,F], w2t [F,H]
    so every SBUF tile load is a natural contiguous 2D slice (no on-device
    transposes anywhere).
  - On device, each core:
      * computes router logits for all tokens (xT-tile as stationary operand,
        gate.T as moving), softmax + top-2 + renorm along the free dim; the
        softmax denominator cancels in the top-2 renormalization, so the
        per-token weight for expert e is exp_e * [exp_e >= v2] / (v1 + v2).
      * runs its expert's MLP densely over all tokens in transposed space:
        h1T/h3T [F-part, T-col] = w1t/w3t-tile.T @ xT, g = silu(h1T)*h3T,
        out [T-part, H-col] = gT-tile.T @ w2t  (g is already in the exact
        stationary layout GEMM3 wants).
      * scales out rows by its expert's per-token routing weight (a
        per-partition scalar in this layout) and writes a [T,H] partial.
  - Host sums the 8 partials (the "all-reduce") and reshapes to [B,S,H].
  - Per-core column-0 trick: each core's gate copy has its columns permuted
    so that column 0 is its own expert -> one SPMD program, no dynamic
    indexing.

Tiling: tokens in chunks of TC=1024; F processed in NQ=4 quarters with an
SBUF fp32 accumulator for the GEMM3 partial sums so that the g-chunk and the
w2 tiles both stay small enough for SBUF.
"""

from contextlib import ExitStack

import ml_dtypes
import numpy as np

import concourse.bacc as bacc
import concourse.tile as tile
from concourse import mybir
from concourse.bass_utils import run_bass_kernel_spmd

P = 128
AF = mybir.ActivationFunctionType
OP = mybir.AluOpType
AX = mybir.AxisListType
F32 = mybir.dt.float32


def build_moe_nc(T=4096, H=1024, F=3584, E=8, TC=1024, NQ=4, silu_native=True,
                 mm_bf16=True):
    """Build the single-core SPMD program. Returns the compiled Bacc."""
    HT = H // P          # contraction k-tiles for GEMM1/2
    FT = F // P          # f tiles
    FQ = FT // NQ        # f-tiles per quarter
    NCH = T // TC        # token chunks
    # moving-operand slices of the token chunk (fp32/bf16 max free dim 512)
    nw_slices = []
    off = 0
    while off < TC:
        w = min(512, TC - off)
        nw_slices.append((off, w))
        off += w
    TH = len(nw_slices)
    # 128-token tiles per chunk, with an optional 64-token tail tile
    ttiles = []
    toff = 0
    while toff < TC:
        th_ = min(P, TC - toff)
        ttiles.append((toff, th_))
        toff += th_
    NTT = len(ttiles)
    HW = min(512, H)     # output H column group width
    HC = H // HW
    assert FT % NQ == 0 and T % TC == 0 and TC % 64 == 0 and H % HW == 0

    MDT = mybir.dt.bfloat16 if mm_bf16 else F32
    nc = bacc.Bacc("TRN2", target_bir_lowering=False, debug=False)
    xt_d = nc.dram_tensor("xt", [H, T], F32, kind="ExternalInput").ap()
    gwt_d = nc.dram_tensor("gwt", [H, E], F32, kind="ExternalInput").ap()
    w1t_d = nc.dram_tensor("w1t", [H, F], MDT, kind="ExternalInput").ap()
    w3t_d = nc.dram_tensor("w3t", [H, F], MDT, kind="ExternalInput").ap()
    w2t_d = nc.dram_tensor("w2t", [F, H], MDT, kind="ExternalInput").ap()
    out_d = nc.dram_tensor("out", [T, H], F32, kind="ExternalOutput").ap()

    with tile.TileContext(nc) as tc, ExitStack() as ctx:
        xt_pool = ctx.enter_context(tc.tile_pool(name="xt", bufs=HT))
        xtb_pool = ctx.enter_context(tc.tile_pool(name="xtb", bufs=HT))
        gw_pool = ctx.enter_context(tc.tile_pool(name="gw", bufs=HT))
        w13_pool = ctx.enter_context(tc.tile_pool(name="w13", bufs=2 * HT))
        w2_pool = ctx.enter_context(tc.tile_pool(name="w2", bufs=FQ * HC))
        g_pool = ctx.enter_context(tc.tile_pool(name="g", bufs=FQ))
        acc_pool = ctx.enter_context(tc.tile_pool(name="acc", bufs=NTT * HC))
        tmp_pool = ctx.enter_context(tc.tile_pool(name="tmp", bufs=4))
        rt_pool = ctx.enter_context(tc.tile_pool(name="rt", bufs=16))
        sc_pool = ctx.enter_context(tc.tile_pool(name="sc", bufs=2 * NTT))
        ob_pool = ctx.enter_context(tc.tile_pool(name="ob", bufs=4))
        ps12 = ctx.enter_context(tc.tile_pool(name="ps12", bufs=6, space="PSUM"))
        ps3 = ctx.enter_context(tc.tile_pool(name="ps3", bufs=2, space="PSUM"))

        # ---- PE warm-up: dense dummy matmuls while the first DMAs land.
        # The HAM clock gate releases (1.2 -> 2.4 GHz) only after a sustained
        # busy window; burn it on zeros during the initial transfer instead
        # of on the first real tiles.
        wu_w = tmp_pool.tile([P, P], MDT, tag="wu")
        wu_x = tmp_pool.tile([P, 512], MDT, tag="wu2")
        nc.vector.memset(wu_w[:], 0.0)
        nc.vector.memset(wu_x[:], 0.0)
        wu_ps = ps3.tile([P, 512], F32, tag="ps3")
        for i in range(72):
            nc.tensor.matmul(wu_ps[:], wu_w[:], wu_x[:],
                             start=(i == 0), stop=(i == 71))

        # gate weights resident for the whole kernel
        gw_tiles = []
        for h in range(HT):
            t_ = gw_pool.tile([P, E], F32, tag="gw")
            nc.sync.dma_start(out=t_[:], in_=gwt_d[h * P:(h + 1) * P, :])
            gw_tiles.append(t_)

        for c in range(NCH):
            t0 = c * TC
            # ---- xT chunk [H, TC] as HT tiles of [P, TC]
            xts = []
            xtb = []
            for h in range(HT):
                t_ = xt_pool.tile([P, TC], F32, tag="xt")
                nc.sync.dma_start(out=t_[:], in_=xt_d[h * P:(h + 1) * P, t0:t0 + TC])
                xts.append(t_)
                if mm_bf16:
                    tb = xtb_pool.tile([P, TC], MDT, tag="xtb")
                    nc.gpsimd.tensor_copy(tb[:], t_[:])
                    xtb.append(tb)
                else:
                    xtb.append(t_)

            # ---- routing for this chunk: per 128-token tile
            scales = []
            for tt, (to, th_) in enumerate(ttiles):
                psr = ps3.tile([th_, E], F32, tag="ps3")
                for h in range(HT):
                    nc.tensor.matmul(
                        psr[:], xts[h][:, to:to + th_], gw_tiles[h][:],
                        start=(h == 0), stop=(h == HT - 1))
                lg = rt_pool.tile([th_, E], F32, tag="rt")
                nc.scalar.copy(lg[:], psr[:])
                nm = rt_pool.tile([th_, 1], F32, tag="rt")
                nc.vector.tensor_reduce(nm[:], lg[:], AX.X, OP.max, negate=True)
                ex = rt_pool.tile([th_, E], F32, tag="rt")
                nc.scalar.activation(ex[:], lg[:], AF.Exp, bias=nm[:])
                v1 = rt_pool.tile([th_, 1], F32, tag="rt")
                nc.vector.tensor_reduce(v1[:], ex[:], AX.X, OP.max)
                ltm = rt_pool.tile([th_, E], F32, tag="rt")
                nc.vector.tensor_scalar(ltm[:], ex[:], v1[:], None, OP.is_lt)
                e2 = rt_pool.tile([th_, E], F32, tag="rt")
                nc.vector.tensor_tensor(e2[:], ex[:], ltm[:], OP.mult)
                v2 = rt_pool.tile([th_, 1], F32, tag="rt")
                nc.vector.tensor_reduce(v2[:], e2[:], AX.X, OP.max)
                den = rt_pool.tile([th_, 1], F32, tag="rt")
                nc.vector.tensor_tensor(den[:], v1[:], v2[:], OP.add)
                rd = rt_pool.tile([th_, 1], F32, tag="rt")
                nc.vector.reciprocal(rd[:], den[:])
                # column 0 is this core's expert; weight = ex0*[ex0>=v2]/(v1+v2)
                ge = rt_pool.tile([th_, 1], F32, tag="rt")
                nc.vector.tensor_scalar(ge[:], ex[:, 0:1], v2[:], None, OP.is_ge)
                w0 = rt_pool.tile([th_, 1], F32, tag="rt")
                nc.vector.tensor_tensor(w0[:], ex[:, 0:1], ge[:], OP.mult)
                sc = sc_pool.tile([th_, 1], F32, tag="sc")
                nc.vector.tensor_tensor(sc[:], w0[:], rd[:], OP.mult)
                scales.append(sc)

            acc_tiles = {}
            for q in range(NQ):
                # ---- quarter weight loads: w1/w3 [P, FQ*P] per h-tile
                w1q, w3q = [], []
                for h in range(HT):
                    t1 = w13_pool.tile([P, FQ * P], MDT, tag="w13")
                    nc.sync.dma_start(
                        out=t1[:],
                        in_=w1t_d[h * P:(h + 1) * P, q * FQ * P:(q + 1) * FQ * P])
                    w1q.append(t1)
                    t3 = w13_pool.tile([P, FQ * P], MDT, tag="w13")
                    nc.sync.dma_start(
                        out=t3[:],
                        in_=w3t_d[h * P:(h + 1) * P, q * FQ * P:(q + 1) * FQ * P])
                    w3q.append(t3)
                w2q = []
                for fq in range(FQ):
                    f = q * FQ + fq
                    row = []
                    for hcol in range(HC):
                        t2 = w2_pool.tile([P, HW], MDT, tag="w2")
                        nc.sync.dma_start(
                            out=t2[:],
                            in_=w2t_d[f * P:(f + 1) * P, hcol * HW:(hcol + 1) * HW])
                        row.append(t2)
                    w2q.append(row)

                # ---- GEMM1/2: h1T/h3T [P(F), NW] + silu*mul -> g tiles [P, TC]
                gq = []
                for fq in range(FQ):
                    p1 = [ps12.tile([P, w], F32, tag="ps12", name=f"p1_{c}_{q}_{fq}_{th}")
                          for th, (o, w) in enumerate(nw_slices)]
                    p3 = [ps12.tile([P, w], F32, tag="ps12", name=f"p3_{c}_{q}_{fq}_{th}")
                          for th, (o, w) in enumerate(nw_slices)]
                    for h in range(HT):
                        lw = w1q[h][:, fq * P:(fq + 1) * P]
                        for th, (o, w) in enumerate(nw_slices):
                            nc.tensor.matmul(
                                p1[th][:], lw, xtb[h][:, o:o + w],
                                start=(h == 0), stop=(h == HT - 1))
                    for h in range(HT):
                        lw = w3q[h][:, fq * P:(fq + 1) * P]
                        for th, (o, w) in enumerate(nw_slices):
                            nc.tensor.matmul(
                                p3[th][:], lw, xtb[h][:, o:o + w],
                                start=(h == 0), stop=(h == HT - 1))
                    gt = g_pool.tile([P, TC], MDT, tag="g")
                    for th, (o, w) in enumerate(nw_slices):
                        tmp = tmp_pool.tile([P, w], F32, tag="tmp")
                        if silu_native:
                            nc.scalar.activation(tmp[:], p1[th][:], AF.Silu)
                        else:
                            # CoreSim has no Silu; sigmoid then explicit mul
                            sg = tmp_pool.tile([P, w], F32, tag="tmp")
                            nc.scalar.activation(sg[:], p1[th][:], AF.Sigmoid)
                            nc.vector.tensor_tensor(tmp[:], sg[:], p1[th][:], OP.mult)
                        nc.vector.tensor_tensor(
                            gt[:, o:o + w], tmp[:], p3[th][:], OP.mult)
                    gq.append(gt)

                # ---- GEMM3: out[T-part, H-col] partial over this quarter's F
                for tt, (to, th_) in enumerate(ttiles):
                    for hcol in range(HC):
                        po = ps3.tile([th_, HW], F32, tag="ps3")
                        for fq in range(FQ):
                            nc.tensor.matmul(
                                po[:], gq[fq][:, to:to + th_], w2q[fq][hcol][:],
                                start=(fq == 0), stop=(fq == FQ - 1))
                        if q == 0:
                            at = acc_pool.tile([th_, HW], F32, tag="acc")
                            acc_tiles[(tt, hcol)] = at
                            if NQ == 1:
                                ob = ob_pool.tile([th_, HW], F32, tag="ob")
                                nc.vector.tensor_scalar(
                                    ob[:], po[:], scales[tt][:], None, OP.mult)
                                nc.sync.dma_start(
                                    out=out_d[t0 + to:t0 + to + th_,
                                              hcol * HW:(hcol + 1) * HW],
                                    in_=ob[:])
                            else:
                                nc.scalar.copy(at[:], po[:])
                        else:
                            at = acc_tiles[(tt, hcol)]
                            nc.vector.tensor_tensor(at[:], po[:], at[:], OP.add)
                            if q == NQ - 1:
                                ob = ob_pool.tile([th_, HW], F32, tag="ob")
                                nc.vector.tensor_scalar(
                                    ob[:], at[:], scales[tt][:], None, OP.mult)
                                nc.sync.dma_start(
                                    out=out_d[t0 + to:t0 + to + th_,
                                              hcol * HW:(hcol + 1) * HW],
                                    in_=ob[:])

    nc.compile()
    return nc


def make_in_maps(hidden_states, gate_w, w1, w2, w3, mm_bf16=True):
    """Shard/transpose FULL inputs into per-core in_maps (expert-parallel)."""
    B, S, H = hidden_states.shape
    E = gate_w.shape[0]
    wdt = ml_dtypes.bfloat16 if mm_bf16 else np.float32
    x2 = np.asarray(hidden_states, dtype=np.float32).reshape(-1, H)
    xt = np.ascontiguousarray(x2.T)
    gt = np.asarray(gate_w, dtype=np.float32).T  # [H, E]
    in_maps = []
    for e in range(E):
        perm = [(e + j) % E for j in range(E)]
        in_maps.append({
            "xt": xt,
            "gwt": np.ascontiguousarray(gt[:, perm]),
            "w1t": np.ascontiguousarray(np.asarray(w1[e], dtype=np.float32).T).astype(wdt),
            "w3t": np.ascontiguousarray(np.asarray(w3[e], dtype=np.float32).T).astype(wdt),
            "w2t": np.ascontiguousarray(np.asarray(w2[e], dtype=np.float32).T).astype(wdt),
        })
    return in_maps


_NC_CACHE = {}


def _get_nc(key, **kw):
    if key not in _NC_CACHE:
        _NC_CACHE[key] = build_moe_nc(**kw)
    return _NC_CACHE[key]


def _host_top2_idx(x2, gate_w):
    """Token index list per expert (host copy of the routing, for sharding).

    The device recomputes the routing weights itself; this only decides
    which (token, expert) pairs each core works on.
    """
    logits = x2.astype(np.float32) @ gate_w.astype(np.float32).T
    order = np.argsort(-logits, axis=1, kind="stable")[:, :2]
    E = gate_w.shape[0]
    return [np.nonzero((order == e).any(axis=1))[0] for e in range(E)]


def kernel(hidden_states, gate_w, w1, w2, w3, _trace=False, _trace_kwargs=None):
    B, S, H = hidden_states.shape
    E = gate_w.shape[0]
    T = B * S
    x2 = np.asarray(hidden_states, dtype=np.float32).reshape(T, H)
    idx = _host_top2_idx(x2, gate_w)
    cmax = max(len(i) for i in idx)
    cpad = max(512, -(-cmax // 64) * 64)

    if cpad <= 2048:
        # sparse path: each core gets only its expert's tokens (padded)
        nc = _get_nc(("sparse", cpad), T=cpad, TC=cpad, NQ=4)
        xt = np.ascontiguousarray(x2.T)
        base = make_in_maps(hidden_states, gate_w, w1, w2, w3)
        in_maps = []
        for e in range(E):
            xg = np.zeros((H, cpad), dtype=np.float32)
            xg[:, :len(idx[e])] = xt[:, idx[e]]
            m = dict(base[e])
            m["xt"] = xg
            in_maps.append(m)
        res = run_bass_kernel_spmd(
            nc, in_maps, list(range(E)), trace=_trace, **(_trace_kwargs or {}))
        kernel.last_results = res
        out = np.zeros((T, H), dtype=np.float32)
        for e, r in enumerate(res.results):
            out[idx[e]] += r["out"][:len(idx[e])]
    else:
        # dense fallback (pathological routing imbalance)
        nc = _get_nc(("dense",), T=T, TC=1024, NQ=4)
        in_maps = make_in_maps(hidden_states, gate_w, w1, w2, w3)
        res = run_bass_kernel_spmd(
            nc, in_maps, list(range(E)), trace=_trace, **(_trace_kwargs or {}))
        kernel.last_results = res
        out = np.zeros((T, H), dtype=np.float32)
        for r in res.results:
            out += r["out"]
    return out.reshape(B, S, H).astype(hidden_states.dtype)
